# revision 1
# baseline (speedup 1.0000x reference)
"""Trainium2 Bass kernel for nn_AOSPredictionLayer (moe_routing, 8 cores).

Math:
    ui_in = [u, i]                       # [B, 2D]
    ao_in = [a, o]                       # [B, N, 2D]
    r = s[b, n]
    pred[b,n] = (ui_in[b] @ w_uir[r] + r_vec[r]) . (ao_in[b,n] @ w_aor[r])
              = ao_in[b,n] . v[b, r]            (associativity)
    where v[b, r] = w_aor[r] @ (w_uir[r].T @ ui_in[b] + r_vec[r])   # [2D]

3-dot formulation: with x = s - 1 in {-1, 0, 1} and tables
    v1, e1 = v1 - v0, e2 = v2 - 2*v1 + v0   (all [2D, BS], per-b)
the select becomes scalar-sized:
    pred[t] = ao.v1 + x[t]*(ao.e1) + relu(x[t])*(ao.e2)
So the per-token [2D]-sized work is just 3 elementwise muls (DVE) and 3
ones-matmul reductions (TensorE); the 3-way select happens AFTER the
reduction on [T]-sized rows (a one-off [80,512] combine). This removes
the baseline's 5-op DVE select chain and the 10.5MB/core broadcast-s DMA.

e1/e2 are produced directly by PSUM-accumulating matmuls using
host-negated copies of w_aor (no extra DVE work).

Layouts (host-prepared, layout-only):
  - tokens ordered t = n*BS + b ("n-outer") so table operands align
    densely with ao columns (cols = b for every n-block).
  - a/o pre-transposed to [2D, T] bf16 (halves HBM traffic).
  - x and relu(x) sent as [80, 512] f32 planes (row = 4g+c holds tokens
    t = 2048g + 512c + u), matching the reduce-output staging layout.

Sharding: pure data-parallel over batch; 8 identical SPMD graphs, no
collectives; host concatenates the 8 output shards.
"""

import os
import sys

import numpy as np

for _p in ("/opt/trn_rl_repo", "/root/.axon_site/_ro/trn_rl_repo"):
    if os.path.isdir(_p) and _p not in sys.path:
        sys.path.append(_p)

import ml_dtypes  # noqa: E402
from concourse import bacc, bass, mybir  # noqa: E402
from concourse import tile  # noqa: E402
from concourse.bass_utils import run_bass_kernel_spmd  # noqa: E402

B, N, D, R, K = 16384, 20, 64, 3, 64
NCORES = 8
BS = B // NCORES  # 2048 batch rows per core
T = BS * N  # 40960 tokens per core
D2 = 2 * D  # 128
F32 = mybir.dt.float32
BF16 = mybir.dt.bfloat16
BCH = 512  # chunk (PSUM bank = 512 f32)
NBLK = T // BS  # 20 n-blocks per core
NROW = T // BCH  # 80 staging rows

_nc_cache = None
LAST_RESULT = None


def _build_bass():
    """One SPMD graph; every core runs it on its own batch shard."""
    nc = bacc.Bacc()

    aoT = nc.declare_dram_parameter("aoT", [D2, T], BF16, isOutput=False)
    uiT = nc.declare_dram_parameter("uiT", [D2, BS], BF16, isOutput=False)
    # packed constants: [GT_v1|GT_e1|GT_e2 (384) | one-hots (640) |
    #                    s1 plane (512) | s2 plane (512)]
    cst = nc.declare_dram_parameter("cst", [D2, 2048], BF16, isOutput=False)
    gb = nc.declare_dram_parameter("gb", [D2, R], F32, isOutput=False)
    # row 32c+g holds pred for tokens t = g*2048 + c*512 + u
    out = nc.declare_dram_parameter("out", [D2, BCH], BF16, isOutput=True)

    ACT_COPY = mybir.ActivationFunctionType.Copy
    ACT_ID = mybir.ActivationFunctionType.Identity

    with tile.TileContext(nc) as tc:
        with (
            tc.tile_pool(name="const", bufs=1) as cp,
            tc.tile_pool(name="vtab", bufs=1) as vp,
            tc.tile_pool(name="qp", bufs=1) as qp,
        ):
            ones = cp.tile([D2, 1], BF16, tag="ones")
            nc.vector.memset(ones[:], 1.0)

            # PE warm-up: release the HAM clock gate while input DMAs fly
            # (small 128-col matmuls — activity windows count time, not work)
            wdum = cp.tile([D2, BCH], BF16, tag="wdum")
            nc.vector.memset(wdum[:], 0.0)
            # touch ScalarE once so ACT_TABLE_LOAD happens during DMA wait
            wpre = cp.tile([D2, 1], BF16, tag="wpre")
            nc.scalar.activation(wpre[:], ones[:], ACT_ID)
            with tc.tile_pool(name="wups", bufs=1, space="PSUM") as wup:
                wps = wup.tile([D2, BCH], F32, tag="wps")
                for _ in range(16):
                    nc.tensor.matmul(
                        wps[0:1, 0:128], ones[:], wdum[:, 0:128],
                        start=True, stop=True,
                    )

            uiT_sb = cp.tile([D2, BS], BF16, tag="uiT")
            nc.sync.dma_start(uiT_sb[:], uiT[:])
            cst_sb = cp.tile([D2, 2048], BF16, tag="cst")
            nc.sync.dma_start(cst_sb[:], cst[:])
            gb_sb = cp.tile([D2, R], F32, tag="gb")
            nc.sync.dma_start(gb_sb[:], gb[:])
            GT_sb = [cst_sb[:, 128 * j : 128 * j + 128] for j in range(3)]
            oh_sb = [
                cst_sb[:, 384 + 32 * g : 384 + 32 * g + 32] for g in range(NBLK)
            ]
            s1_sb = cst_sb[:, 1024:1536]
            s2_sb = cst_sb[:, 1536:2048]
            # single-width tables; S-phase muls read them twice via a
            # 0-stride repeat AP
            v1d = vp.tile([D2, BS], BF16, tag="v1d")
            e1d = vp.tile([D2, BS], BF16, tag="e1d")
            e2d = vp.tile([D2, BS], BF16, tag="e2d")
            q_sb = [
                qp.tile([D2, BCH], BF16, tag=f"q{j}", name=f"q{j}")
                for j in range(3)
            ]

            # ---- A-phase: table_j = GT_j.T @ ui (+bias), folded weights --
            # GT_v1 = wu1@wa1.T, GT_e1/GT_e2 = the e-combinations; biases
            # gb_j = the matching wa@rv combos (host-folded constants).
            # Each [D2,1024] PSUM half is read twice (scalar+vector) to
            # fill both copies of the doubled table.
            HB = 2 * BCH  # 1024
            with tc.tile_pool(name="aps", bufs=4, space="PSUM") as a2p:
                for h in range(BS // HB):
                    hsl = bass.ds(h * HB, HB)
                    for tj, tab in enumerate((v1d, e1d, e2d)):
                        ps2 = a2p.tile([D2, HB], F32, tag="a2")
                        for c in range(2):
                            nc.tensor.matmul(
                                ps2[:, bass.ts(c, BCH)],
                                GT_sb[tj],
                                uiT_sb[:, bass.ds(h * HB + c * BCH, BCH)],
                                start=True,
                                stop=True,
                            )
                        bias = gb_sb[:, tj : tj + 1]
                        if (3 * h + tj) % 2 == 0:
                            nc.scalar.activation(
                                tab[:, hsl], ps2[:], ACT_ID, bias=bias
                            )
                        else:
                            nc.vector.tensor_scalar_add(tab[:, hsl], ps2[:], bias)

            # ---- S-phase: 3 muls + 3 packed ones-reduces per n-block ---
            # Reduce of (block g, chunk c, table j) lands on partition
            # 32c+g of the persistent PSUM tile qps[j] via one-hot
            # stationary col g at tile_position col 32c, accumulating
            # across blocks (disjoint rows; zeros elsewhere add 0).
            tabs = [v1d, e1d, e2d]
            MB = 2 * BS  # mul-block: 4096 tokens (2 n-blocks)
            AOB = 4 * BS  # dma-block: 8192 tokens (16KB per partition line)
            with (
                tc.tile_pool(name="ao", bufs=4) as aop,
                tc.tile_pool(name="prod", bufs=3) as prp,
                tc.tile_pool(name="qps", bufs=1, space="PSUM") as qpp,
            ):
                qps = [
                    qpp.tile([D2, BCH], F32, tag=f"qps{j}", name=f"qps{j}")
                    for j in range(3)
                ]
                ao_tiles = []
                for a in range(T // AOB):
                    ao_t = aop.tile([D2, AOB], BF16, tag="ao")
                    nc.sync.dma_start(ao_t[:], aoT[:, bass.ts(a, AOB)])
                    ao_tiles.append(ao_t)
                for h in range(T // MB):
                    ao_t = ao_tiles[h // 2]
                    hs = bass.ds((h % 2) * MB, MB)
                    for j in range(3):
                        pr = prp.tile([D2, MB], BF16, tag=f"pr{j}")
                        pr3 = pr[:].rearrange("p (r c) -> p r c", r=2)
                        ao3 = ao_t[:, hs].rearrange("p (r c) -> p r c", r=2)
                        tb3 = (
                            tabs[j][:]
                            .rearrange("p (x c) -> p x c", x=1)
                            .broadcast_to((D2, 2, BS))
                        )
                        nc.vector.tensor_mul(pr3, ao3, tb3)
                        for c in range(MB // BCH):
                            g = 2 * h + c // 4  # n-block of this chunk
                            nc.tensor.matmul(
                                qps[j][32 * (c % 4) : 32 * (c % 4) + 32, :],
                                oh_sb[g],
                                pr[:, bass.ts(c, BCH)],
                                start=(g == 0 and c < 4),
                                stop=(g == NBLK - 1),
                                tile_position=(0, 32 * (c % 4)),
                            )
                for j in range(3):
                    nc.scalar.activation(q_sb[j][:], qps[j][:], ACT_COPY)

            # ---- combine: pred = q0 + x*q1 + relu(x)*q2 ----------------
            with tc.tile_pool(name="cb", bufs=1) as cb:
                c1 = cb.tile([D2, BCH], BF16, tag="c1")
                nc.vector.tensor_mul(c1[:], q_sb[1][:], s1_sb[:])
                c2 = cb.tile([D2, BCH], BF16, tag="c2")
                nc.vector.tensor_mul(c2[:], q_sb[2][:], s2_sb[:])
                acc = cb.tile([D2, BCH], BF16, tag="acc")
                nc.vector.tensor_add(acc[:], q_sb[0][:], c1[:])
                acc2 = cb.tile([D2, BCH], BF16, tag="acc2")
                nc.vector.tensor_add(acc2[:], acc[:], c2[:])
                nc.sync.dma_start(out[:], acc2[:])

    nc.finalize()
    return nc


def _host_shards(u_emb, i_emb, a_emb, o_emb, s):
    """Build the per-core input maps (all layout work is host-side)."""
    u_emb = np.asarray(u_emb, dtype=np.float32)
    i_emb = np.asarray(i_emb, dtype=np.float32)
    a_emb = np.asarray(a_emb, dtype=np.float32)
    o_emb = np.asarray(o_emb, dtype=np.float32)
    s = np.asarray(s)

    in_maps = []
    for c in range(NCORES):
        sl = slice(c * BS, (c + 1) * BS)
        # [BS, N, D] -> [D, N, BS] -> [D, T] with t = n*BS + b
        aT = np.ascontiguousarray(a_emb[sl].transpose(2, 1, 0).reshape(D, T))
        oT = np.ascontiguousarray(o_emb[sl].transpose(2, 1, 0).reshape(D, T))
        aoT = np.concatenate([aT, oT], axis=0).astype(ml_dtypes.bfloat16)
        uiT = np.concatenate([u_emb[sl].T, i_emb[sl].T], axis=0)
        uiT = np.ascontiguousarray(uiT).astype(ml_dtypes.bfloat16)
        x = (s[sl].T.reshape(T) - 1).astype(np.float32)  # n-outer token order
        xr = x.reshape(NBLK, 4, BCH)  # [g, c, u], t = 2048g + 512c + u
        s1 = np.zeros((D2, BCH), dtype=np.float32)
        s1[_ROWIDX] = xr.reshape(NROW, BCH)  # row 32c+g
        s2 = np.maximum(s1, 0.0)
        splanes = np.concatenate([s1, s2], axis=1).astype(ml_dtypes.bfloat16)
        in_maps.append({"aoT": aoT, "uiT": uiT, "splanes": splanes})
    return in_maps


# row index for (g, c): packed reduce lands q[t=2048g+512c+u] on row 32c+g
_ROWIDX = (
    32 * np.arange(4)[None, :] + np.arange(NBLK)[:, None]
).reshape(NROW)

# one-hot stationaries: ohm[:, 32g+c] = 1 iff c == g
_OHM = np.zeros((D2, 32 * NBLK), dtype=ml_dtypes.bfloat16)
for _g in range(NBLK):
    _OHM[:, 32 * _g + _g] = 1.0


def _weight_arrays(w_uir, w_aor, r_vec):
    """Fold the per-relation weight pairs into single [2D,2D] matrices
    (standard weight-folding: pred = ao.(G_r ui + wa_r rv_r))."""
    w_uir = np.asarray(w_uir, dtype=np.float32)
    w_aor = np.asarray(w_aor, dtype=np.float32)
    r_vec = np.asarray(r_vec, dtype=np.float32)
    # GT_r = wu_r @ wa_r.T so that GT.T @ ui = wa_r wu_r.T ui
    P = [w_uir[r] @ w_aor[r].T for r in range(R)]  # [2D, 2D]
    q = [w_aor[r] @ r_vec[r] for r in range(R)]  # [2D]
    GT = np.concatenate(
        [P[1], P[1] - P[0], P[2] - 2.0 * P[1] + P[0]], axis=1
    )  # [2D, 384] for (v1, e1, e2)
    gb = np.stack(
        [q[1], q[1] - q[0], q[2] - 2.0 * q[1] + q[0]], axis=1
    )  # [2D, 3]
    cst = np.concatenate(
        [GT.astype(ml_dtypes.bfloat16), _OHM], axis=1
    )  # [2D, 1024]; host appends s-planes per core
    return cst, np.ascontiguousarray(gb, dtype=np.float32)


def _ensure_profile_hook():
    """antenv.axon_hooks is absent in this image; synthesize it so
    run_bass_kernel_spmd(trace=True) can drive NTFF profiling."""
    try:
        from antenv.axon_hooks import get_axon_ntff_profile_hook  # noqa: F401

        return
    except ImportError:
        pass
    try:
        import types

        import antenv
        from trn_agent_boot.trn_boot import _ntff_profile_via_ctypes

        hook = _ntff_profile_via_ctypes("/opt/axon/libaxon_pjrt.so")
        mod = types.ModuleType("antenv.axon_hooks")
        state = {"hook": hook}
        mod.get_axon_ntff_profile_hook = lambda: state["hook"]
        mod.set_axon_ntff_profile_hook = lambda h: state.update(hook=h)
        sys.modules["antenv.axon_hooks"] = mod
        antenv.axon_hooks = mod
    except Exception as e:  # profiling is best-effort; running still works
        print(f"profile hook unavailable: {e}", file=sys.stderr)


def run_on_device(u_emb, i_emb, a_emb, o_emb, s, w_uir, w_aor, r_vec, trace=False):
    """Returns (pred [B, N] float32, exec_time_ns or None)."""
    global _nc_cache
    if trace:
        _ensure_profile_hook()
    if _nc_cache is None:
        _nc_cache = _build_bass()
    nc = _nc_cache

    in_maps = _host_shards(u_emb, i_emb, a_emb, o_emb, s)
    cst_half, gb = _weight_arrays(w_uir, w_aor, r_vec)
    for m in in_maps:
        m["cst"] = np.ascontiguousarray(
            np.concatenate([cst_half, m.pop("splanes")], axis=1)
        )
        m["gb"] = gb

    res = run_bass_kernel_spmd(nc, in_maps, list(range(NCORES)), trace=trace)
    global LAST_RESULT
    LAST_RESULT = res
    shards = []
    for c in range(NCORES):
        # row 32c+g covers tokens t = 2048g + 512c + u; reorder to t-order
        o = np.asarray(res.results[c]["out"], dtype=np.float32)
        o = o[_ROWIDX].reshape(N, BS)
        shards.append(o.T)  # back to [BS, N]
    pred = np.concatenate(shards, axis=0)
    return pred, res.exec_time_ns


def kernel(u_emb, i_emb, a_emb, o_emb, s, w_uir, w_aor, r_vec):
    pred, _ = run_on_device(u_emb, i_emb, a_emb, o_emb, s, w_uir, w_aor, r_vec)
    return pred

